# revision 26
# baseline (speedup 1.0000x reference)
"""Compressed-KV GPT-2 attention block on 8 TRN2 NeuronCores — v2.

Sharding: batch x head-group. Core c: batch b = c//4, heads 4*(c%4)..+4
(= 2 head-pairs). Transposed-activation layout ([dim, seq] on partitions);
each core emits a partial c_proj output^T; host sums 4 partials per batch.

v2 structural changes over the 264us baseline:
  - rank-32 factorization: k' = (x @ (w_k wk_c)) @ (wk_d/sqrt(hd)) and
    v_dec = (x @ (w_v wv_c)) @ wv_d, so the qkv matmul emits 32-wide
    compressed k/v columns (512 total vs 768) and the decompress runs as
    tiny reused-stationary matmuls (k': K=32 row-tiled 2x2; v: one
    block-diagonal rhs decompresses+transposes both heads per key chunk).
  - head-pair row-tiling: even head on array rows 0-63, odd head on rows
    64-127 (implicit tile_position from base partitions), so the K=64
    score matmuls for a pair run concurrently in the PE array.
  - paired PSUM tiles [128,1024] (2 banks): qkv/score/proj matmuls fill 2
    banks that drain with ONE wide DVE/ACT op, halving per-instruction
    overhead (exp esp.: ACTIVATE pays a 352-cycle fixed cost).
  - softmax normalize: DVE reciprocal of the PSUM den row -> DRAM bounce
    -> broadcast DMA -> one DVE multiply (replaces the 4-DMA reshape
    chain per iteration); rec-path DMAs ride the idle GpSimd queue.
  - c_proj for seq-block sb is emitted inside attention of sb+1 so its
    matmuls/stores overlap; only the last block's projection is a tail.
"""

import sys

if "/opt/trn_rl_repo" not in sys.path:
    sys.path.insert(0, "/opt/trn_rl_repo")

import numpy as np
import ml_dtypes

BF16 = ml_dtypes.bfloat16

B, S, D = 2, 2048, 1024
H, hd, C = 16, 64, 32
NCORES = 8
SB = 512
NSB = S // SB      # 4 seq blocks of 512
NKT = S // 128     # 16 key tiles of 128
DC = D // 128      # 8 contraction chunks for qkv

_cache = {}


def _build():
    import concourse.bacc as bacc
    import concourse.tile as tile
    import concourse.mybir as mybir

    dt = mybir.dt
    f32, bf16 = dt.float32, dt.bfloat16
    Exp = mybir.ActivationFunctionType.Exp
    mult = mybir.AluOpType.mult

    nc = bacc.Bacc("TRN2", target_bir_lowering=False, debug=False, num_devices=NCORES)

    hidden_t = nc.dram_tensor("hidden_t", [D, S], bf16, kind="ExternalInput")
    w_qkv = nc.dram_tensor("w_qkv", [D, 512], bf16, kind="ExternalInput")
    b_qkv = nc.dram_tensor("b_qkv", [128, 4], f32, kind="ExternalInput")
    wkd2_d = nc.dram_tensor("wkd2", [2, 64, 128], bf16, kind="ExternalInput")
    wvd2_d = nc.dram_tensor("wvd2", [2, 64, 128], bf16, kind="ExternalInput")
    w_proj = nc.dram_tensor("w_proj", [2, 128, D], bf16, kind="ExternalInput")
    maskd = nc.dram_tensor("maskd", [128, 128], bf16, kind="ExternalInput")
    out_t = nc.dram_tensor("out_t", [D, S], bf16, kind="ExternalOutput")

    with tile.TileContext(nc) as tc:
        with (
            tc.tile_pool(name="persist", bufs=1) as pp,
            tc.tile_pool(name="work", bufs=2) as wp,
            tc.tile_pool(name="epool", bufs=6) as ep,
            tc.tile_pool(name="dscr", bufs=4, space="DRAM") as dr,
            tc.tile_pool(name="psb", bufs=3, space="PSUM") as psb,
            tc.tile_pool(name="pss", bufs=2, space="PSUM") as pss,
        ):
            # ---- loads: qkv weights + hidden first (consumption order) ----
            bias = pp.tile([128, 4], f32, name="bias")
            nc.sync.dma_start(bias[:], b_qkv.ap())
            # three big strided DMAs instead of 24 small ones: per-transfer
            # queue spacing (~1.3us) was gating the first qkv matmul chain
            wqall = pp.tile([128, DC * 512], bf16, name="wqall")
            wsrc = w_qkv.ap().rearrange("(d p) c -> p d c", p=128)
            wdst = wqall[:].rearrange("p (d c) -> p d c", c=512)
            nc.scalar.dma_start(wdst[:, 0:4, :], wsrc[:, 0:4, :])
            nc.scalar.dma_start(wdst[:, 4:8, :], wsrc[:, 4:8, :])
            wq = [wqall[:, d * 512:(d + 1) * 512] for d in range(DC)]
            hTall = pp.tile([128, DC * S], bf16, name="hTall")
            hsrc = hidden_t.ap().rearrange("(d p) s -> p d s", p=128)
            hdst = hTall[:].rearrange("p (d s) -> p d s", s=S)
            # sbp0 halves first (they gate the first qkv chains), sbp1 after
            for sbp in range(2):
                ssl = slice(sbp * 1024, (sbp + 1) * 1024)
                nc.sync.dma_start(hdst[:, 0:2, ssl], hsrc[:, 0:2, ssl])
                nc.gpsimd.dma_start(hdst[:, 2:4, ssl], hsrc[:, 2:4, ssl])
                nc.sync.dma_start(hdst[:, 4:6, ssl], hsrc[:, 4:6, ssl])
                nc.gpsimd.dma_start(hdst[:, 6:8, ssl], hsrc[:, 6:8, ssl])
            hT = [hTall[:, d * S:(d + 1) * S] for d in range(DC)]
            maskt = pp.tile([128, 128], bf16, name="maskt")
            nc.scalar.dma_start(maskt[:], maskd.ap())
            wkd2, wvd2, wpj = [], [], []
            for p in range(2):
                t = pp.tile([128, 128], bf16, name=f"wkd2_{p}")
                nc.scalar.dma_start(t[0:64, :], wkd2_d.ap()[p])
                wkd2.append(t)
                t2 = pp.tile([128, 128], bf16, name=f"wvd2_{p}")
                nc.scalar.dma_start(t2[64:128, :], wvd2_d.ap()[p])
                wvd2.append(t2)
                t3 = pp.tile([128, D], bf16, name=f"wpj{p}")
                nc.scalar.dma_start(t3[:], w_proj.ap()[p])
                wpj.append(t3)

            # ---- qkv^T: m-blocks [q0|q1],[q2|q3],[kc0|kc1|vc0|vc1]x2 ----
            # sb 0-1 run up front as pairs; sb 2-3 are emitted one sb at a
            # time as PE filler inside the qsb0/qsb1 attention blocks.
            qq = [pp.tile([128, S], bf16, name=f"qq{p}") for p in range(2)]
            KC = [pp.tile([128, S], bf16, name=f"kc{p}") for p in range(2)]
            dests = qq + KC
            kk = [pp.tile([128, S], bf16, name=f"kk{p}") for p in range(2)]
            vdo = [pp.tile([128, NKT * 130], bf16, name=f"vdo{p}") for p in range(2)]
            for p in range(2):
                nc.vector.memset(vdo[p][:], 1.0)
            ones64 = pp.tile([128, 64], bf16, name="ones64")
            nc.vector.memset(ones64[:], 1.0)

            def qkv_unit(mb, sbs):
                ps = psb.tile([128, 1024], f32, tag="ps2", name="psq")
                for (j, sb) in enumerate(sbs):
                    for d in range(DC):
                        nc.tensor.matmul(
                            ps[:, j * 512:(j + 1) * 512],
                            wq[d][:, mb * 128:(mb + 1) * 128],
                            hT[d][:, sb * 512:(sb + 1) * 512],
                            start=(d == 0),
                            stop=(d == DC - 1),
                        )
                nw = 512 * len(sbs)
                nc.vector.tensor_scalar_add(
                    out=dests[mb][:, sbs[0] * 512:sbs[0] * 512 + nw],
                    in0=ps[:, 0:nw],
                    scalar1=bias[:, mb:mb + 1],
                )

            def decomp(sb):
                sl = slice(sb * SB, (sb + 1) * SB)
                for p in range(2):
                    psK = pss.tile([128, 512], f32, tag="ps1", name="psK")
                    nc.tensor.matmul(
                        psK[0:64, :], wkd2[p][0:32, 0:64], KC[p][0:32, sl]
                    )
                    nc.tensor.matmul(
                        psK[64:128, :], wkd2[p][32:64, 64:128], KC[p][32:64, sl]
                    )
                    nc.vector.tensor_copy(kk[p][:, sl], psK[:])
                for p in range(2):
                    psC = pss.tile([128, 512], f32, tag="ps1", name="psC")
                    for cch in range(4):
                        st = 4 * sb + cch
                        nc.tensor.matmul(
                            psC[:, cch * 128:(cch + 1) * 128],
                            KC[p][64:128, st * 128:(st + 1) * 128],
                            wvd2[p][64:128, :],
                        )
                    src = psC[:].rearrange("p (c w) -> p c w", w=128)
                    dst = vdo[p][:, 4 * sb * 130:(4 * sb + 4) * 130].rearrange(
                        "p (c w) -> p c w", w=130
                    )
                    nc.vector.tensor_copy(dst[:, :, 0:64], src[:, :, 0:64])
                    nc.vector.tensor_copy(dst[:, :, 65:129], src[:, :, 64:128])

            for mb in (2, 3, 0, 1):
                qkv_unit(mb, (0, 1))
            decomp(0)
            decomp(1)

            # PE filler emitted after each early attention block: sb2/sb3
            # qkv units + their decompress, keeping the PE busy while ACT
            # works through qsb0/qsb1 exponentials.
            fillers = {
                (0, 0): lambda: [qkv_unit(2, (2,)), qkv_unit(3, (2,))],
                (0, 1): lambda: [qkv_unit(0, (2,)), qkv_unit(1, (2,))],
                (1, 0): lambda: [decomp(2), qkv_unit(2, (3,)), qkv_unit(3, (3,))],
                (1, 1): lambda: [qkv_unit(0, (3,)), qkv_unit(1, (3,)), decomp(3)],
            }

            # ---- attention (qsb-outer so c_proj overlaps) + merge ----
            attn = [pp.tile([128, S], bf16, name=f"attn{p}") for p in range(2)]

            proj3_state = []

            def emit_proj3_phase1():
                # head-pair-0 matmuls depend only on the earlier-finishing
                # attn[0], so they fill the PE while the final normalize
                # chains for attn[1] drain (emitted BEFORE that normalize
                # so nothing DVE-dependent blocks them in the PE FIFO)
                sl = slice((NSB - 1) * SB, NSB * SB)
                for mbp in range(3):
                    psP = psb.tile([128, 1024], f32, tag="ps2", name="psP")
                    proj3_state.append(psP)
                    for j in range(2):
                        mb = 2 * mbp + j
                        nc.tensor.matmul(
                            psP[:, j * 512:(j + 1) * 512],
                            wpj[0][:, mb * 128:(mb + 1) * 128],
                            attn[0][:, sl], start=True, stop=False,
                        )

            def emit_proj3_phase2():
                sl = slice((NSB - 1) * SB, NSB * SB)
                for mbp in range(4):
                    if mbp == 3:
                        psP = psb.tile([128, 1024], f32, tag="ps2", name="psP")
                        proj3_state.append(psP)
                    psP = proj3_state[mbp]
                    for j in range(2):
                        mb = 2 * mbp + j
                        if mbp == 3:
                            nc.tensor.matmul(
                                psP[:, j * 512:(j + 1) * 512],
                                wpj[0][:, mb * 128:(mb + 1) * 128],
                                attn[0][:, sl], start=True, stop=False,
                            )
                        nc.tensor.matmul(
                            psP[:, j * 512:(j + 1) * 512],
                            wpj[1][:, mb * 128:(mb + 1) * 128],
                            attn[1][:, sl], start=False, stop=True,
                        )
                    stage = wp.tile(
                        [128, 1024], bf16, tag="stage", bufs=3, name="stage"
                    )
                    nc.scalar.activation(
                        stage[:, 0:512], psP[:, 0:512],
                        mybir.ActivationFunctionType.Copy,
                    )
                    nc.vector.tensor_copy(stage[:, 512:1024], psP[:, 512:1024])
                    dst = out_t.ap()[2 * mbp * 128:(2 * mbp + 2) * 128, sl]
                    nc.sync.dma_start(
                        dst.rearrange("(j p) s -> p j s", p=128),
                        stage[:].rearrange("p (j s) -> p j s", s=512),
                    )

            def emit_proj(sb, last=False):
                sl = slice(sb * SB, (sb + 1) * SB)
                for mbp in range(4):
                    psP = psb.tile([128, 1024], f32, tag="ps2", name="psP")
                    for j in range(2):
                        mb = 2 * mbp + j
                        for p in range(2):
                            nc.tensor.matmul(
                                psP[:, j * 512:(j + 1) * 512],
                                wpj[p][:, mb * 128:(mb + 1) * 128],
                                attn[p][:, sl],
                                start=(p == 0),
                                stop=(p == 1),
                            )
                    stage = wp.tile([128, 1024], bf16, tag="stage", bufs=3, name="stage")
                    nc.vector.tensor_copy(stage[:], psP[:])
                    dst = out_t.ap()[2 * mbp * 128:(2 * mbp + 2) * 128, sl]
                    nc.sync.dma_start(
                        dst.rearrange("(j p) s -> p j s", p=128),
                        stage[:].rearrange("p (j s) -> p j s", s=512),
                    )

            for qsb in range(NSB):
                qsl = slice(qsb * SB, (qsb + 1) * SB)
                nkb = 4 * qsb + 4
                for hp in range(2):
                    pso_e = pss.tile([128, 512], f32, tag="ps1", name="psoE")
                    pso_o = pss.tile([128, 512], f32, tag="ps1", name="psoO")

                    def emit_attnv(e2_e, e2_o, kbA, kbB, c0A, c0B):
                        for (e2, pso, oh) in ((e2_e, pso_e, 0), (e2_o, pso_o, 65)):
                            for (jj, kb, c0) in ((0, kbA, c0A), (1, kbB, c0B)):
                                nc.tensor.matmul(
                                    pso[0:65, c0:512],
                                    vdo[hp][:, kb * 130 + oh:kb * 130 + oh + 65],
                                    e2[:, jj * 512 + c0:(jj + 1) * 512],
                                    start=(kb == 0),
                                    stop=(kb == nkb - 1),
                                )

                    prev = None
                    for kbp in range(nkb // 2):
                        kbA, kbB = 2 * kbp, 2 * kbp + 1
                        rA, rB = kbA - 4 * qsb, kbB - 4 * qsb
                        c0A, c0B = max(rA, 0) * 128, max(rB, 0) * 128
                        psS_e = psb.tile([128, 1024], f32, tag="ps2", name="psSe")
                        psS_o = psb.tile([128, 1024], f32, tag="ps2", name="psSo")
                        for (jj, kb, c0) in ((0, kbA, c0A), (1, kbB, c0B)):
                            ksl = slice(kb * 128, (kb + 1) * 128)
                            qs2 = slice(qsb * SB + c0, (qsb + 1) * SB)
                            nc.tensor.matmul(
                                psS_e[:, jj * 512 + c0:(jj + 1) * 512],
                                kk[hp][0:64, ksl], qq[hp][0:64, qs2],
                            )
                            nc.tensor.matmul(
                                psS_o[:, jj * 512 + c0:(jj + 1) * 512],
                                kk[hp][64:128, ksl], qq[hp][64:128, qs2],
                            )
                        e2_e = ep.tile([128, 1024], bf16, tag="e2", name="e2e")
                        e2_o = ep.tile([128, 1024], bf16, tag="e2", name="e2o")
                        band = rA >= 0
                        for (e2, psS) in ((e2_e, psS_e), (e2_o, psS_o)):
                            if not band:
                                nc.scalar.activation(e2[:], psS[:], Exp)
                            else:
                                # one strided ACTIVATE covers both halves from
                                # c0B; the A-only strip [c0A:c0B] goes separate
                                ev = e2[:].rearrange("p (j w) -> p j w", w=512)
                                pv = psS[:].rearrange("p (j w) -> p j w", w=512)
                                nc.scalar.activation(
                                    ev[:, :, c0B:512], pv[:, :, c0B:512], Exp
                                )
                                nc.scalar.activation(
                                    e2[:, c0A:c0B], psS[:, c0A:c0B], Exp
                                )
                                for lo in (c0A, 512 + c0B):
                                    nc.vector.tensor_tensor(
                                        e2[:, lo:lo + 128], e2[:, lo:lo + 128],
                                        maskt[:], mult,
                                    )
                        if prev is not None:
                            emit_attnv(*prev)
                        prev = (e2_e, e2_o, kbA, kbB, c0A, c0B)
                        if hp == 1 and kbp == 1 and qsb > 0:
                            emit_proj(qsb - 1)
                    emit_attnv(*prev)

                    # normalize: num/den; den is pso row 64 (ones col of vdo).
                    # First evacuate pso -> SBUF so the PSUM slot frees at
                    # once (the PE must not wait on this chain), then bounce
                    # den through DRAM into [128,4] for a cheap reciprocal
                    # (DVE recip is ~8 cyc per free-elem, serial per lane).
                    # The two head-chains run on different DMA queues
                    # (even->gpsimd, odd->sync) so they pipeline in parallel.
                    # For the last qsb the DMA-chain latency (~2us/hop) would
                    # sit fully exposed in the tail, so normalize there via a
                    # K=1 PE broadcast of den + one DVE divide instead.
                    tail = qsb == NSB - 1
                    if tail and hp == 1:
                        emit_proj3_phase1()
                    for (pso, even) in ((pso_o, False), (pso_e, True)):
                        eng = nc.sync if even else nc.gpsimd
                        nsb = wp.tile([128, 512], bf16, tag="nsb", bufs=3, name="nsb")
                        nc.vector.tensor_copy(nsb[0:65, :], pso[0:65, :])
                        if tail and even and hp == 1:
                            # tail: den-broadcast via K=1 PE matmul, then a
                            # (slow but single-hop) DVE reciprocal — runs in
                            # parallel with the odd head's DMA chain above
                            psB = pss.tile([128, 512], f32, tag="ps1", name="psB")
                            nc.tensor.matmul(
                                psB[0:64, :], ones64[64:65, :], nsb[64:65, :]
                            )
                            bcr = wp.tile([64, 512], bf16, tag="bc", name="bcr")
                            with nc.allow_low_precision(reason="softmax recip bf16"):
                                nc.vector.reciprocal(bcr[:], psB[0:64, :])
                            bc = bcr[:]
                            op = mult
                        else:
                            dend = dr.tile([512], bf16, tag="dend", name="dend")
                            eng.dma_start(dend[:], nsb[64:65, :])
                            denc = wp.tile([128, 4], bf16, tag="denc", name="denc")
                            eng.dma_start(
                                denc[:], dend[:].rearrange("(p j) -> p j", p=128)
                            )
                            recc = wp.tile([128, 4], bf16, tag="recc", name="recc")
                            with nc.allow_low_precision(reason="softmax recip bf16"):
                                nc.vector.reciprocal(recc[:], denc[:])
                            recd = dr.tile([512], bf16, tag="recd", name="recd")
                            eng.dma_start(
                                recd[:].rearrange("(p j) -> p j", p=128), recc[:]
                            )
                            bct = wp.tile([64, 512], bf16, tag="bc", name="bc")
                            eng.dma_start(
                                bct[:], recd[:].unsqueeze(0).to_broadcast([64, 512])
                            )
                            bc = bct[:]
                            op = mult
                        if even:
                            nc.vector.tensor_tensor(
                                attn[hp][0:64, qsl], nsb[0:64, :], bc, op
                            )
                        else:
                            atmp = wp.tile([64, 512], bf16, tag="atmp", name="atmp")
                            nc.vector.tensor_tensor(
                                atmp[:], nsb[0:64, :], bc, op
                            )
                            eng.dma_start(attn[hp][64:128, qsl], atmp[:])

                    if (qsb, hp) in fillers:
                        fillers[(qsb, hp)]()
            emit_proj3_phase2()

    nc.compile()
    return nc


def _prep_inputs(hidden_states, w_attn, b_attn, wk_c, wv_c, wk_d, wv_d, w_proj):
    """Per-core input maps (host-side shard + rank-32 fold + bf16 cast).

    k' = k @ (wk_c wk_d / sqrt(hd)) factors as (x @ (w_k wk_c)) @ (wk_d/8):
    the 32-wide compressed projections fold into w_qkv columns, the 32->64
    decompressors ship as tiny per-pair matrices. Same for v with wv_*.
    """
    f64 = np.float64
    hidden_T = [np.ascontiguousarray(hidden_states[b].T).astype(BF16) for b in range(B)]
    wq_h = lambda h: w_attn[:, h * hd:(h + 1) * hd]
    wkcf = lambda h: (w_attn[:, D + h * hd:D + (h + 1) * hd].astype(f64)
                      @ wk_c[h].astype(f64)).astype(np.float32)
    wvcf = lambda h: (w_attn[:, 2 * D + h * hd:2 * D + (h + 1) * hd].astype(f64)
                      @ wv_c[h].astype(f64)).astype(np.float32)
    bq_h = lambda h: b_attn[h * hd:(h + 1) * hd]
    bkc = lambda h: (b_attn[D + h * hd:D + (h + 1) * hd].astype(f64)
                     @ wk_c[h].astype(f64)).astype(np.float32)
    bvc = lambda h: (b_attn[2 * D + h * hd:2 * D + (h + 1) * hd].astype(f64)
                     @ wv_c[h].astype(f64)).astype(np.float32)
    kk_ = np.arange(128).reshape(128, 1)
    cg = np.arange(128).reshape(1, 128)
    mask = np.ascontiguousarray((kk_ <= cg).astype(BF16))
    in_maps = []
    for c in range(NCORES):
        b = c // 4
        hs4 = [4 * (c % 4) + i for i in range(4)]
        pairs = [(hs4[0], hs4[1]), (hs4[2], hs4[3])]
        wcols, bcols = [], []
        for (he, ho) in pairs:
            wcols.append(np.concatenate([wq_h(he), wq_h(ho)], 1))
            bcols.append(np.concatenate([bq_h(he), bq_h(ho)]))
        for (he, ho) in pairs:
            wcols.append(np.concatenate([wkcf(he), wkcf(ho), wvcf(he), wvcf(ho)], 1))
            bcols.append(np.concatenate([bkc(he), bkc(ho), bvc(he), bvc(ho)]))
        w_qkv_l = np.concatenate(wcols, 1).astype(BF16)          # [1024, 512]
        b_qkv_l = np.stack(bcols, 1).astype(np.float32)          # [128, 4]
        wkd2 = np.zeros((2, 64, 128), np.float32)
        wvd2 = np.zeros((2, 64, 128), np.float32)
        for p, (he, ho) in enumerate(pairs):
            wkd2[p, 0:32, 0:64] = wk_d[he] / np.sqrt(hd)
            wkd2[p, 32:64, 64:128] = wk_d[ho] / np.sqrt(hd)
            wvd2[p, 0:32, 0:64] = wv_d[he]
            wvd2[p, 32:64, 64:128] = wv_d[ho]
        wpj_l = np.stack([
            np.concatenate([w_proj[he * hd:(he + 1) * hd, :],
                            w_proj[ho * hd:(ho + 1) * hd, :]], 0)
            for (he, ho) in pairs
        ])
        in_maps.append({
            "hidden_t": hidden_T[b],
            "w_qkv": w_qkv_l,
            "b_qkv": b_qkv_l,
            "wkd2": wkd2.astype(BF16),
            "wvd2": wvd2.astype(BF16),
            "w_proj": wpj_l.astype(BF16),
            "maskd": mask,
        })
    return in_maps


def kernel(
    hidden_states,
    w_attn,
    b_attn,
    w_proj,
    b_proj,
    wk_c,
    wv_c,
    wk_d,
    wv_d,
    _trace=False,
):
    from concourse.bass_utils import run_bass_kernel_spmd

    if "nc" not in _cache:
        _cache["nc"] = _build()
    nc = _cache["nc"]

    in_maps = _prep_inputs(
        np.asarray(hidden_states),
        np.asarray(w_attn),
        np.asarray(b_attn),
        np.asarray(wk_c),
        np.asarray(wv_c),
        np.asarray(wk_d),
        np.asarray(wv_d),
        np.asarray(w_proj),
    )
    res = run_bass_kernel_spmd(
        nc, in_maps, core_ids=list(range(NCORES)), trace=_trace
    )
    out = np.empty((B, S, D), np.float32)
    for b in range(B):
        acc = np.zeros((D, S), np.float32)
        for c in range(4 * b, 4 * b + 4):
            acc += res.results[c]["out_t"].astype(np.float32)
        out[b] = acc.T + np.asarray(b_proj, np.float32)
    if _trace:
        _cache["last_exec_time_ns"] = res.exec_time_ns
        _cache["last_results"] = res
    return out
